# revision 59
# baseline (speedup 1.0000x reference)
"""Multi-head attention (12 heads, N=4096, C=768) on 8 TRN2 NeuronCores.

Sharding: 8 cores = 4 head-groups x 2 sequence halves.
  core c: heads 3*(c%4) .. 3*(c%4)+2, query rows half (c//4).
Each core computes K/V projections for its 3 heads over the FULL sequence
(inputs are passed with the core's query half rotated to the front, which is
legal because softmax+PV is permutation-invariant along the key axis), Q for
its 2048 query rows, eager attention, and a partial output projection. Host
sums the 4 head-group partials per sequence half and adds the bias terms.

Engine plan (cost-model driven):
 - Projections run in bf16 (fp8 x/weights cost ~2% output error - too much);
   Q^T/K^T are then quantized to fp8 and DMA-remapped into the [32, 2, n]
   DoubleRow layout (head h packed at partition offset 32h) so the S = K^T Q
   matmuls run as fp8 DoubleRow at 0.5 cycles/column.
 - S^T tiles [128k x 1024q] in PSUM; softmax exp is split ~60/40 between the
   Act engine (exp, reads PSUM directly, writes fp8 SBUF) and GpSimd
   (base^x pow after a DVE PSUM->SBUF stage) to break the single-engine
   activation wall.
 - PV accumulates transposed [q, d] (full 128-partition outputs) as mixed
   fp8-stationary x bf16-moving matmuls, with a ones-column in V producing
   the softmax denominator for free; normalization happens on the DVE during
   PSUM evacuation via a stride-0 broadcast multiply. One accumulator per
   128-query group, each owning a full PSUM bank (start_tensor_calc zeroes
   the whole 2KB zero-region).
 - Attention output is PE-transposed back to [d, q] (bf16) for the output
   projection; all non-gating projections, remaps, PV of the previous block
   and deferred output projections are woven into the kt loops at kp
   granularity so no engine queue gets a long homogeneous burst.
 - PE warm-up matmuls hold the p-state at full clock through the ramp; big
   late-need DMAs are WAR-gated behind the attention-gating remap transfers
   (the tile scheduler is readiness-ordered).

Bias algebra (exact): bk drops out of softmax entirely; bv contributes
bv @ Wo to every output row (added on host with bo); bq is folded into Q.
"""

import math

import numpy as np
import ml_dtypes

import concourse.bass as bass
from concourse import bacc
import concourse.tile as tile
import concourse.mybir as mybir
from concourse.bass_utils import run_bass_kernel_spmd
from concourse.masks import make_identity

P = 128
C = 768                    # hidden
NSEQ = 4096                # sequence length
HPC = 3                    # heads per core
HD = 64                    # head dim
HW = HPC * HD              # 192, projection width per core
QB = 2048                  # query rows per core
QBLK = 1024                # query block per S tile
KT = NSEQ // P             # 32 key tiles
KP = KT // 2               # 16 key-tile pairs
NCH = C // P               # 6 contraction chunks (3 DoubleRow pairs)
SC = 0.125                 # softmax scale (1/sqrt(HD))
F8 = mybir.dt.float8e4
BF16 = mybir.dt.bfloat16
F32 = mybir.dt.float32
AF = mybir.ActivationFunctionType
ALU = mybir.AluOpType
DR = mybir.MatmulPerfMode.DoubleRow

_CACHE = {}

# set by test.py to capture profiling info
TRACE = False
LAST_RESULT = None


def _build():
    nc = bacc.Bacc("TRN2")

    xT = nc.dram_tensor("xT", [C, NSEQ], BF16, kind="ExternalInput")
    wq01 = nc.dram_tensor("wq01", [C, P], BF16, kind="ExternalInput")
    wk01 = nc.dram_tensor("wk01", [C, P], BF16, kind="ExternalInput")
    wkq2 = nc.dram_tensor("wkq2", [C, P], BF16, kind="ExternalInput")
    wv = nc.dram_tensor("wv", [C, HW], BF16, kind="ExternalInput")
    wo01 = nc.dram_tensor("wo01", [P, C], BF16, kind="ExternalInput")
    wo2 = nc.dram_tensor("wo2", [HD, C], BF16, kind="ExternalInput")
    bq01 = nc.dram_tensor("bq01", [P, 1], F32, kind="ExternalInput")
    bkq2 = nc.dram_tensor("bkq2", [P, 1], F32, kind="ExternalInput")
    out = nc.dram_tensor("out", [QB, C], BF16, kind="ExternalOutput")

    with tile.TileContext(nc) as tc:
        with (
            tc.tile_pool(name="const", bufs=1) as const,
            tc.tile_pool(name="proj", bufs=1) as proj,
            tc.tile_pool(name="ptp", bufs=2) as ptp,
            tc.tile_pool(name="ssbp", bufs=3) as ssbp,
            tc.tile_pool(name="ostp", bufs=4) as ostp,
            tc.tile_pool(name="psp", bufs=3, space="PSUM") as psp,
            tc.tile_pool(name="psw", bufs=2, space="PSUM") as psw,
        ):
            # ---- constants / inputs (weights first: they gate the first
            # projection matmuls and the DMA_ENGINES device serializes) ----
            wq01_s = const.tile([P, NCH, P], BF16)
            nc.sync.dma_start(wq01_s[:], wq01[:].rearrange("(c p) m -> p c m", p=P))
            wk01_s = const.tile([P, NCH, P], BF16)
            nc.sync.dma_start(wk01_s[:], wk01[:].rearrange("(c p) m -> p c m", p=P))
            bq01_s = const.tile([P, 1], F32)
            nc.sync.dma_start(bq01_s[:], bq01[:])
            bkq2_s = const.tile([P, 1], F32)
            nc.sync.dma_start(bkq2_s[:], bkq2[:])
            xt = const.tile([P, NCH, NSEQ], BF16)
            src = xT[:].rearrange("(c p) n -> p c n", p=P)
            for i in range(4):
                nc.sync.dma_start(xt[:, :, i * 512:(i + 1) * 512],
                                  src[:, :, i * 512:(i + 1) * 512])
            # wkq2/wv/wo are not needed until well into block 0 — they are
            # WAR-gated below so they can't hog the DMA device ahead of the
            # attention-gating remaps
            wkq2_s = const.tile([P, NCH, P], BF16)
            wv_s = const.tile([P, NCH, HW], BF16)
            wo01_s = const.tile([P, C], BF16)
            wo2_s = const.tile([HD, C], BF16)
            ident = const.tile([P, P], BF16)
            make_identity(nc, ident[:])
            base_bc = const.tile([P, 1], F32)
            nc.vector.memset(base_bc[:], math.exp(SC))
            # warm the Exp activation table while inputs stream in
            warm = const.tile([1, 1], F32)
            nc.vector.memset(warm[:], 0.0)
            nc.scalar.activation(warm[:], warm[:], AF.Exp)
            # keep the PE continuously busy while inputs stream in: the PE
            # p-state ramps (0.65 -> 1.2 -> 2.4 GHz over ~3us of continuous
            # work), and any idle resets it — without this the whole gating
            # projection prefix runs at half clock
            wps = psw.tile([P, P], F32, tag="w")
            for _ in range(55):
                nc.tensor.matmul(wps[:], ident[:], ident[:], start=True, stop=True)

            # ---- persistent projection outputs ----
            K8_01 = proj.tile([P, NSEQ], F8)     # K^T heads 0,1 (d on parts)
            KQ2_8 = proj.tile([P, NSEQ], F8)     # K^T h2 (parts 0:64) + Q^T h2 (64:128)
            Q8_01 = proj.tile([P, QB], F8)       # Q^T heads 0,1
            KDR = proj.tile([96, 2, NSEQ], F8)   # S stationary, head h at partitions 32h
            QDR = proj.tile([96, 2, QB], F8)     # S moving, head h at partitions 32h
            V_bf = proj.tile([P, KT, HPC, HD + 1], BF16)  # V + den ones col
            A_sb = proj.tile([P, QB // P, HPC, HD], BF16)  # [q, qs, h, d]
            AT01 = proj.tile([P, QB], BF16)      # A^T heads 0,1
            AT2 = proj.tile([HD, QB], BF16)      # A^T head 2
            rcp = proj.tile([P, HPC, 2, 8, 1], F32)
            acc1 = proj.tile([P, 8, HD + 1], F32)   # last block's kt0-15 PV

            nc.vector.memset(V_bf[:, :, :, HD:HD + 1], 1.0)

            # ---- projections (bf16: exact q,k,v feeding the fp8
            # attention core; fp8 x/W cost ~2% output error, too much) ----
            def proj_mms(ps_ap, w_s, cols, n0, nn):
                for j in range(NCH):
                    nc.tensor.matmul(
                        ps_ap, w_s[:, j, 0:cols], xt[:, j, n0:n0 + nn],
                        start=(j == 0), stop=(j == NCH - 1),
                    )

            # Evacuations alternate DVE / Act (Identity shares the exp act
            # table, so no table reloads) to halve the projection-phase wall.
            def emit_q01(nt):
                psq = psw.tile([P, 512], F32, tag="w")
                proj_mms(psq[:], wq01_s, P, nt * 512, 512)
                dst = Q8_01[:, nt * 512:(nt + 1) * 512]
                if nt % 2 == 0:
                    nc.vector.tensor_scalar(dst, psq[:], bq01_s[:], None, ALU.add)
                else:
                    nc.scalar.activation(dst, psq[:], AF.Identity, bias=bq01_s[:])

            def emit_k01(nt):
                psk = psw.tile([P, 512], F32, tag="w")
                proj_mms(psk[:], wk01_s, P, nt * 512, 512)
                dst = K8_01[:, nt * 512:(nt + 1) * 512]
                if nt % 2 == 0:
                    nc.vector.tensor_copy(dst, psk[:])
                else:
                    nc.scalar.activation(dst, psk[:], AF.Copy)

            def emit_kq2(nt):
                psm = psw.tile([P, 512], F32, tag="w")
                proj_mms(psm[:], wkq2_s, P, nt * 512, 512)
                dst = KQ2_8[:, nt * 512:(nt + 1) * 512]
                if nt % 2 == 0:
                    nc.vector.tensor_scalar(dst, psm[:], bkq2_s[:], None, ALU.add)
                else:
                    nc.scalar.activation(dst, psm[:], AF.Identity, bias=bkq2_s[:])

            def rk(h, qtr):
                # KDR head h, cols [1024*qtr, 1024*(qtr+1)) — 2 DMAs
                nlo, nn = qtr * 1024, 1024
                ksrc, base = (K8_01, 64 * h) if h < 2 else (KQ2_8, 0)
                for t in range(2):
                    nc.sync.dma_start(
                        KDR[32 * h:32 * h + 32, t, nlo:nlo + nn],
                        ksrc[base + 32 * t:base + 32 * t + 32, nlo:nlo + nn])

            def rq(h, half):
                nlo, nn = half * 1024, 1024
                qsrc, base = (Q8_01, 64 * h) if h < 2 else (KQ2_8, 64)
                for t in range(2):
                    nc.sync.dma_start(
                        QDR[32 * h:32 * h + 32, t, nlo:nlo + nn],
                        qsrc[base + 32 * t:base + 32 * t + 32, nlo:nlo + nn])

            # Minimal gating prefix: 4 projection tiles + 4 remap DMAs
            # unblock attention on (qb0, h0, kt 0..7). Every remaining
            # projection/remap trickles through block 0/1's kt slots below
            # (each DMA instruction costs ~650ns serial HWDGE time, so the
            # ramp can't afford a big batch of them).
            for nt in range(2):
                emit_q01(nt)
            for nt in range(2):
                emit_k01(nt)
            rk(0, 0)
            rq(0, 0)
            for nt in range(2, 4):
                emit_k01(nt)
            # The dummy reads of xt's back region (paired with KDR/QDR reads)
            # create WAR dependencies so these big x DMAs queue on the shared
            # DMA engines only AFTER the attention-gating remap transfers
            # (the tile scheduler is readiness-ordered, not program-ordered).
            scrap = const.tile([1, 2], F32)
            for gate_ap, dep in ((xt[0:1, 0, 2048:2050], KDR[0:1, 0, 1024:1026]),
                                 (xt[0:1, 0, 3072:3074], KDR[0:1, 0, 1024:1026]),
                                 (wkq2_s[0:1, 0, 0:2], KDR[0:1, 0, 3072:3074]),
                                 (wv_s[0:1, 0, 0:2], KDR[0:1, 0, 3072:3074]),
                                 (wo01_s[0:1, 0:2], KDR[0:1, 0, 3072:3074]),
                                 (wo2_s[0:1, 0:2], KDR[0:1, 0, 3072:3074])):
                nc.vector.tensor_tensor(scrap[:], gate_ap, dep, ALU.bypass)
            nc.sync.dma_start(wkq2_s[:], wkq2[:].rearrange("(c p) m -> p c m", p=P))
            for i in range(4):
                nc.sync.dma_start(xt[:, :, 2048 + i * 512:2048 + (i + 1) * 512],
                                  src[:, :, 2048 + i * 512:2048 + (i + 1) * 512])
            nc.sync.dma_start(wv_s[:], wv[:].rearrange("(c p) m -> p c m", p=P))
            nc.sync.dma_start(wo01_s[:], wo01[:])
            nc.sync.dma_start(wo2_s[:], wo2[:])

            # ---- V projection (emitted interleaved into block 0's kt loop;
            # PV first needs V8 only one block later) ----
            def emit_vproj(kt):
                psv = psw.tile([P, HW], F32, tag="w")
                for j in range(NCH):
                    nc.tensor.matmul(
                        psv[:], xt[:, j, kt * P:(kt + 1) * P], wv_s[:, j, :],
                        start=(j == 0), stop=(j == NCH - 1),
                    )
                vdst = V_bf[:, kt, :, 0:HD]
                vsrc = psv[:].rearrange("p (h d) -> p h d", d=HD)
                if kt % 2 == 0:
                    nc.vector.tensor_copy(vdst, vsrc)
                else:
                    nc.scalar.activation(vdst, vsrc, AF.Copy)

            # ---- out-projection for one 128-query subtile (deferred) ----
            # tail=True: the S-tile psum ring is free, use it for extra
            # parallelism, and alternate evacuations onto the idle Act engine.
            def emit_transp(qs, tail=False):
                pool, tag = (psp, "s") if tail else (psw, "w")
                t01 = pool.tile([P, P], BF16, tag=tag)
                nc.tensor.transpose(t01[:], A_sb[:, qs, 0:2, :], ident[:])
                at01_dst = AT01[:, qs * P:(qs + 1) * P]
                if tail:
                    nc.scalar.activation(at01_dst, t01[:], AF.Copy)
                else:
                    nc.vector.tensor_copy(at01_dst, t01[:])
                t2 = pool.tile([HD, P], BF16, tag=tag)
                nc.tensor.transpose(t2[:], A_sb[:, qs, 2, :], ident[:])
                nc.vector.tensor_copy(AT2[:, qs * P:(qs + 1) * P], t2[:])

            def emit_po(qs, tail=False):
                pool, tag = (psp, "s") if tail else (psw, "w")
                ost = ostp.tile([P, C], BF16, tag="ost")
                for half in range(2):
                    po = pool.tile([P, 384], F32, tag=tag)
                    nc.tensor.matmul(
                        po[:], AT01[:, qs * P:(qs + 1) * P],
                        wo01_s[:, half * 384:(half + 1) * 384],
                        start=True, stop=False,
                    )
                    nc.tensor.matmul(
                        po[:], AT2[:, qs * P:(qs + 1) * P],
                        wo2_s[:, half * 384:(half + 1) * 384],
                        start=False, stop=True,
                    )
                    dst = ost[:, half * 384:(half + 1) * 384]
                    if tail and half == 1:
                        nc.scalar.activation(dst, po[:], AF.Copy)
                    else:
                        nc.vector.tensor_copy(dst, po[:])
                nc.sync.dma_start(out[qs * P:(qs + 1) * P, :], ost[:])

            def emit_outproj(qs, tail=False):
                emit_transp(qs, tail)
                emit_po(qs, tail)

            # ---- attention ----
            # Software-pipelined: the PV + normalize of block i-1 (and qb0's
            # output projections) are woven INTO block i's kt loop at kp
            # granularity, so no engine queue ever gets a long homogeneous
            # burst (DVE head-of-line blocking killed a coarser version).
            # Each per-qs PV accumulator owns a full PSUM bank: hardware
            # zeroes the whole 2KB zero-region on start_tensor_calc.
            def emit_pv_group(qb, h, pta, qs):
                acc = psw.tile([P, HD + 1], F32, tag="w")
                for kt in range(KT):
                    nc.tensor.matmul(
                        acc[:], pta[:, kt // 2, kt % 2, qs * P:(qs + 1) * P],
                        V_bf[:, kt, h, :],
                        start=(kt == 0), stop=(kt == KT - 1),
                    )
                nc.vector.reciprocal(rcp[:, h, qb, qs, :], acc[:, HD:HD + 1])
                nc.vector.tensor_tensor(
                    A_sb[:, qb * 8 + qs, h, :], acc[:, 0:HD],
                    rcp[:, h, qb, qs, :].broadcast_to([P, HD]), ALU.mult,
                )

            # Deferred projection/remap work distributed over blocks 0/1's kt
            # slots. Deadlines: KDR h0 quarter qtr before block0 kp 4*qtr;
            # KDR/QDR h1 before block 1, h2 before block 2; QDR col-half 1
            # before block 3 (qb1); V8 complete before block 1's kp6 (first
            # PV). The V projection count per slot balances PE load.
            def J(f, *a):
                return lambda: f(*a)

            vq = iter(range(KT))
            SCHED = {
                (0, 0): [J(emit_k01, 4), J(emit_kq2, 0), J(rk, 0, 1)],
                (0, 1): [J(emit_k01, 5), J(emit_kq2, 1)],
                (0, 2): [J(emit_k01, 6), J(emit_kq2, 2)],
                (0, 3): [J(emit_k01, 7), J(emit_kq2, 3), J(rk, 1, 0), J(rk, 1, 1)],
                (0, 4): [J(emit_kq2, 4), J(rk, 0, 2)],
                (0, 5): [J(emit_kq2, 5), J(emit_q01, 2)],
                (0, 6): [J(emit_kq2, 6), J(rk, 0, 3), J(emit_q01, 3)],
                (0, 7): [J(emit_kq2, 7), J(rk, 1, 2), J(rk, 1, 3)],
                (0, 8): [J(rk, 2, 0), J(rk, 2, 1), J(rq, 1, 0)],
                (0, 9): [J(rk, 2, 2), J(rk, 2, 3)] + [J(emit_vproj, next(vq)) for _ in range(3)],
                (0, 10): [J(rq, 2, 0)] + [J(emit_vproj, next(vq)) for _ in range(3)],
                (0, 11): [J(rq, 0, 1), J(rq, 1, 1), J(rq, 2, 1)] + [J(emit_vproj, next(vq)) for _ in range(3)],
                (0, 12): [J(emit_vproj, next(vq)) for _ in range(3)],
                (0, 13): [J(emit_vproj, next(vq)) for _ in range(3)],
                (0, 14): [J(emit_vproj, next(vq)) for _ in range(3)],
                (0, 15): [J(emit_vproj, next(vq)) for _ in range(3)],
                (1, 0): [J(emit_vproj, next(vq)) for _ in range(3)],
                (1, 1): [J(emit_vproj, next(vq)) for _ in range(3)],
                (1, 2): [J(emit_vproj, next(vq)) for _ in range(3)],
                (1, 3): [J(emit_vproj, next(vq)) for _ in range(2)],
            }
            assert next(vq, None) is None

            prev = None
            for bi, (qb, h) in enumerate([(q, hh) for q in range(2) for hh in range(HPC)]):
                pta = ptp.tile([P, KP, 2, QBLK], F8, tag="pt")
                for kp in range(KP):
                    for t in range(2):
                        kt = 2 * kp + t
                        S = psp.tile([P, QBLK], F32, tag="s")
                        for qc in range(4):
                            nc.tensor.matmul(
                                S[:, qc * 256:(qc + 1) * 256],
                                KDR[32 * h:32 * h + 32, :, kt * P:(kt + 1) * P],
                                QDR[32 * h:32 * h + 32, :, qb * QBLK + qc * 256:qb * QBLK + (qc + 1) * 256],
                                start=True, stop=True, perf_mode=DR,
                            )
                        if kt % 5 in (0, 2, 4):
                            nc.scalar.activation(pta[:, kp, t, :], S[:],
                                                 AF.Exp, scale=SC)
                        else:
                            ssb = ssbp.tile([P, QBLK], BF16, tag="ssb")
                            nc.vector.tensor_copy(ssb[:], S[:])
                            nc.gpsimd.tensor_tensor(
                                pta[:, kp, t, :], base_bc[:].broadcast_to([P, QBLK]),
                                ssb[:], ALU.pow,
                            )
                    for job in SCHED.get((bi, kp), ()):
                        job()
                    if prev is not None and 6 <= kp < 14:
                        g = kp - 6
                        emit_pv_group(*prev, g)
                    # qb0's out-projections spread across blocks 3..5
                    if bi == 3 and kp in (7, 11):
                        emit_outproj(1 if kp == 7 else 5)
                    if bi == 4 and kp in (0, 4, 15):
                        emit_outproj([3, 7, 0][(0, 4, 15).index(kp)])
                    if bi == 5 and kp in (0, 2, 4):
                        emit_outproj([2, 4, 6][(0, 2, 4).index(kp)])
                    # last block: first-half PV runs inside its own kt loop
                    if bi == 5 and kp >= 8:
                        qs = kp - 8
                        a1 = psw.tile([P, HD + 1], F32, tag="w")
                        for kt in range(KP):
                            nc.tensor.matmul(
                                a1[:], pta[:, kt // 2, kt % 2, qs * P:(qs + 1) * P],
                                V_bf[:, kt, h, :],
                                start=(kt == 0), stop=(kt == KP - 1),
                            )
                        nc.vector.tensor_copy(acc1[:, qs, :], a1[:])
                prev = (qb, h, pta)
            # tail: second-half PV of the last block, combined with the
            # stored first half, then transposes and the po streams — staged
            # so each engine's queue pipelines across qs
            qb, h, pta = prev
            for qs in range(8):
                a2 = psw.tile([P, HD + 1], F32, tag="w")
                for kt in range(KP, KT):
                    nc.tensor.matmul(
                        a2[:], pta[:, kt // 2, kt % 2, qs * P:(qs + 1) * P],
                        V_bf[:, kt, h, :],
                        start=(kt == KP), stop=(kt == KT - 1),
                    )
                comb = ssbp.tile([P, HD + 1], F32, tag="comb")
                nc.vector.tensor_tensor(comb[:], a2[:], acc1[:, qs, :], ALU.add)
                nc.vector.reciprocal(rcp[:, h, qb, qs, :], comb[:, HD:HD + 1])
                nc.vector.tensor_tensor(
                    A_sb[:, qb * 8 + qs, h, :], comb[:, 0:HD],
                    rcp[:, h, qb, qs, :].broadcast_to([P, HD]), ALU.mult,
                )
                emit_transp(8 + qs, tail=True)
            for qs in range(8, 16):
                emit_po(qs, tail=True)

    if hasattr(nc, "compile"):
        nc.compile()
    return nc


def _get_nc():
    if "nc" not in _CACHE:
        _CACHE["nc"] = _build()
    return _CACHE["nc"]


def kernel(x, Wq, bq, Wk, bk, Wv, bv, Wo, bo):
    global LAST_RESULT
    x = np.asarray(x, dtype=np.float32)
    Wq = np.asarray(Wq, dtype=np.float32)
    Wk = np.asarray(Wk, dtype=np.float32)
    Wv = np.asarray(Wv, dtype=np.float32)
    Wo = np.asarray(Wo, dtype=np.float32)
    bq = np.asarray(bq, dtype=np.float32)
    bv = np.asarray(bv, dtype=np.float32)
    bo = np.asarray(bo, dtype=np.float32)

    B, N, Ch = x.shape
    assert (B, N, Ch) == (1, NSEQ, C)
    bf = ml_dtypes.bfloat16

    xT_full = np.ascontiguousarray(x[0].T)  # [C, N] f32
    xT_rot = np.concatenate([xT_full[:, QB:], xT_full[:, :QB]], axis=1)
    xTb = {0: xT_full.astype(bf), 1: xT_rot.astype(bf)}

    in_maps = []
    for c in range(8):
        qhalf = c // 4
        hbase = HPC * (c % 4)
        cols = slice(hbase * HD, hbase * HD + HW)
        wq_c = Wq[:, cols]
        wk_c = Wk[:, cols]
        bq_c = bq[cols]
        bkq2_v = np.concatenate([np.zeros(HD, np.float32), bq_c[P:HW]])
        in_maps.append({
            "xT": xTb[qhalf],
            "wq01": np.ascontiguousarray(wq_c[:, 0:P]).astype(bf),
            "wk01": np.ascontiguousarray(wk_c[:, 0:P]).astype(bf),
            "wkq2": np.ascontiguousarray(
                np.concatenate([wk_c[:, P:HW], wq_c[:, P:HW]], axis=1)).astype(bf),
            "wv": np.ascontiguousarray(Wv[:, cols]).astype(bf),
            "wo01": np.ascontiguousarray(Wo[cols, :][0:P]).astype(bf),
            "wo2": np.ascontiguousarray(Wo[cols, :][P:HW]).astype(bf),
            "bq01": np.ascontiguousarray(bq_c[0:P].reshape(P, 1)),
            "bkq2": np.ascontiguousarray(bkq2_v.reshape(P, 1)),
        })

    nc = _get_nc()
    res = run_bass_kernel_spmd(nc, in_maps, core_ids=list(range(8)), trace=TRACE)
    LAST_RESULT = res

    out = np.zeros((NSEQ, C), np.float32)
    for c in range(4):
        out[:QB] += res.results[c]["out"].astype(np.float32)
    for c in range(4, 8):
        out[QB:] += res.results[c]["out"].astype(np.float32)
    out += bo + bv @ Wo
    return out.reshape(1, NSEQ, C)
